# revision 6
# baseline (speedup 1.0000x reference)
"""GCN (nn_GraphTransformerNet) Trainium2 kernel, 8-core SPMD.

The reference network is linear (no activation):
    x_{l+1} = A_hat (x_l W_l) + b_l,   out = 1^T x_3 / sqrt(N)
so the sum-pool propagates backward through the layers as per-node scalars:
    u2 = A_hat^T 1, u1 = A_hat^T u2, u0 = A_hat^T u1
    out = ((u0^T X W0 + s1 b0) W1 + s2 b1) W2 + N b2) / sqrt(N)
with s1 = sum(u1), s2 = sum(u2).

Host computes the index-derived per-node weights u0 (pure edge_index math);
the device does the memory-bound work: the u0^T X reduction over all node
features (51 MB streamed, sharded 8 ways), the cross-core AllReduce, and the
dense weight chain + bias terms.
"""
import numpy as np

N = 100000
E = 600000
D = 128
L = 3
N_CORES = 8
P = 128
TILES_PER_CORE = 98                     # 98 * 128 = 12544 nodes per core
N_PAD = N_CORES * TILES_PER_CORE * P    # 100352

_CACHE = {}


def _build_nc():
    import concourse.bacc as bacc
    import concourse.mybir as mybir
    from concourse.tile import TileContext

    nc = bacc.Bacc("TRN2", target_bir_lowering=False, debug=False,
                   num_devices=N_CORES)
    x_in = nc.dram_tensor("x_sh", [P, TILES_PER_CORE * D], mybir.dt.float32,
                          kind="ExternalInput")
    u_in = nc.dram_tensor("u_sh", [P, TILES_PER_CORE], mybir.dt.float32,
                          kind="ExternalInput")
    w_in = nc.dram_tensor("w_chain", [P, L * D], mybir.dt.float32,
                          kind="ExternalInput")
    bias_in = nc.dram_tensor("bias_tot", [P, 1], mybir.dt.float32,
                             kind="ExternalInput")
    out_t = nc.dram_tensor("out", [P, 1], mybir.dt.float32,
                           kind="ExternalOutput")

    with TileContext(nc) as tc:
        with (
            tc.tile_pool(name="misc", bufs=1) as misc,
            tc.tile_pool(name="psum", bufs=1, space="PSUM") as psum,
            tc.tile_pool(name="dram", bufs=1, space="DRAM") as dram,
        ):
            u_sb = misc.tile([P, TILES_PER_CORE], mybir.dt.float32)
            nc.sync.dma_start(u_sb[:], u_in[:, :])
            w_sb = misc.tile([P, L * D], mybir.dt.float32)
            nc.sync.dma_start(w_sb[:], w_in[:, :])
            bias_sb = misc.tile([P, 1], mybir.dt.float32)
            nc.sync.dma_start(bias_sb[:], bias_in[:, :])

            # whole x shard resident in SBUF (50KB/partition), loaded in a
            # few large DMAs so matmuls on early chunks overlap later loads
            x_sb = misc.tile([P, TILES_PER_CORE * D], mybir.dt.float32)
            chunk_tiles = 14
            bounds = list(range(0, TILES_PER_CORE, chunk_tiles)) + [TILES_PER_CORE]
            for lo, hi in zip(bounds[:-1], bounds[1:]):
                nc.sync.dma_start(x_sb[:, lo * D:hi * D], x_in[:, lo * D:hi * D])

            # z = sum_t x_tile^T @ u0_tile  -> [D, 1] accumulated in PSUM
            z_ps = psum.tile([P, 1], mybir.dt.float32)
            for t in range(TILES_PER_CORE):
                nc.tensor.matmul(
                    z_ps[:], lhsT=x_sb[:, t * D:(t + 1) * D], rhs=u_sb[:, t:t + 1],
                    start=(t == 0), stop=(t == TILES_PER_CORE - 1),
                )

            z_sb = misc.tile([P, 1], mybir.dt.float32)
            nc.vector.tensor_copy(z_sb[:], z_ps[:])

            # AllReduce partial z across the 8 cores
            cc_in = dram.tile([P, 1], mybir.dt.float32)
            cc_out = dram.tile([P, 1], mybir.dt.float32)
            nc.gpsimd.dma_start(cc_in[:], z_sb[:])
            nc.gpsimd.collective_compute(
                "AllReduce", mybir.AluOpType.add,
                replica_groups=[list(range(N_CORES))],
                ins=[cc_in.opt()], outs=[cc_out.opt()],
            )
            v_sb = misc.tile([P, 1], mybir.dt.float32)
            nc.gpsimd.dma_start(v_sb[:], cc_out[:])

            # chain: v <- W_l^T v   (gives (z W0 W1 W2)^T), biases folded on host
            cur = v_sb
            for l in range(L):
                ps = psum.tile([P, 1], mybir.dt.float32, tag=f"c{l}")
                nc.tensor.matmul(ps[:], lhsT=w_sb[:, l * D:(l + 1) * D],
                                 rhs=cur[:], start=True, stop=True)
                nxt = misc.tile([P, 1], mybir.dt.float32, tag=f"v{l}")
                nc.vector.tensor_copy(nxt[:], ps[:])
                cur = nxt

            res = misc.tile([P, 1], mybir.dt.float32)
            nc.vector.tensor_add(res[:], cur[:], bias_sb[:])
            nc.sync.dma_start(out_t[:, :], res[:])
    nc.compile()
    return nc


def kernel(edge_index, node_features, Ws, bs):
    from concourse.bass_utils import run_bass_kernel_spmd

    edge_index = np.asarray(edge_index)
    x = np.asarray(node_features, dtype=np.float32)
    Ws = np.asarray(Ws, dtype=np.float32)
    bs = np.asarray(bs, dtype=np.float32)
    src = edge_index[0].astype(np.int64)
    dst = edge_index[1].astype(np.int64)
    n = x.shape[0]

    # ---- host: index-derived propagation weights (scalar per node) ----
    deg = (np.bincount(dst, minlength=n) + 1.0).astype(np.float32)
    dinv = (1.0 / np.sqrt(deg)).astype(np.float32)
    u = np.ones(n, np.float32)
    sums = []
    for _ in range(L):
        t = dinv * u
        u = dinv * (np.bincount(src, weights=t[dst], minlength=n)
                    .astype(np.float32) + t)
        sums.append(float(u.sum()))
    s2, s1 = sums[0], sums[1]
    u0 = u
    sqrt_n = np.sqrt(np.float32(n))

    # bias terms of the backward-substituted output (zero for zero biases)
    bias_total = ((s1 * bs[0]) @ Ws[1] @ Ws[2] + s2 * bs[1] @ Ws[2]
                  + n * bs[2]) / sqrt_n

    # ---- shard inputs across the 8 cores (one vectorized pass) ----
    u0_pad = np.zeros(N_PAD, np.float32)
    u0_pad[:n] = u0 / sqrt_n
    x_pad = np.empty((N_PAD, D), np.float32)
    x_pad[:n] = x
    x_pad[n:] = 0.0

    u_all = np.ascontiguousarray(
        u0_pad.reshape(N_CORES, TILES_PER_CORE, P).transpose(0, 2, 1))
    x_all = np.ascontiguousarray(
        x_pad.reshape(N_CORES, TILES_PER_CORE, P, D)
        .transpose(0, 2, 1, 3)
        .reshape(N_CORES, P, TILES_PER_CORE * D))

    w_chain = np.ascontiguousarray(
        np.concatenate([Ws[l] for l in range(L)], axis=1))  # [128, 3*128]
    bias_col = np.ascontiguousarray(bias_total.reshape(P, 1))
    in_maps = [{
        "x_sh": x_all[c],
        "u_sh": u_all[c],
        "w_chain": w_chain,
        "bias_tot": bias_col,
    } for c in range(N_CORES)]

    if "nc" not in _CACHE:
        _CACHE["nc"] = _build_nc()
    res = run_bass_kernel_spmd(_CACHE["nc"], in_maps, list(range(N_CORES)))
    return res.results[0]["out"].reshape(D).astype(np.float32)


# revision 7
# speedup vs baseline: 1.4707x; 1.4707x over previous
"""GCN (nn_GraphTransformerNet) Trainium2 kernel, 8-core SPMD.

The reference network is linear (no activation):
    x_{l+1} = A_hat (x_l W_l) + b_l,   out = 1^T x_3 / sqrt(N)
so the sum-pool propagates backward through the layers as per-node scalars:
    u2 = A_hat^T 1, u1 = A_hat^T u2, u0 = A_hat^T u1
    out = ((u0^T X W0 + s1 b0) W1 + s2 b1) W2 + N b2) / sqrt(N)
with s1 = sum(u1), s2 = sum(u2).

Host computes the index-derived per-node weights u0 (pure edge_index math);
the device does the memory-bound work: the u0^T X reduction over all node
features (51 MB streamed, sharded 8 ways), the cross-core AllReduce, and the
dense weight chain + bias terms.
"""
import numpy as np

N = 100000
E = 600000
D = 128
L = 3
N_CORES = 8
P = 128
TILES_PER_CORE = 98                     # 98 * 128 = 12544 nodes per core
N_PAD = N_CORES * TILES_PER_CORE * P    # 100352

_CACHE = {}


def _build_nc():
    import concourse.bacc as bacc
    import concourse.mybir as mybir
    from concourse.tile import TileContext

    nc = bacc.Bacc("TRN2", target_bir_lowering=False, debug=False,
                   num_devices=N_CORES)
    x_in = nc.dram_tensor("x_sh", [P, TILES_PER_CORE * D], mybir.dt.float32,
                          kind="ExternalInput")
    u_in = nc.dram_tensor("u_sh", [P, TILES_PER_CORE], mybir.dt.float32,
                          kind="ExternalInput")
    w_in = nc.dram_tensor("w_chain", [P, L * D], mybir.dt.float32,
                          kind="ExternalInput")
    bias_in = nc.dram_tensor("bias_tot", [P, 1], mybir.dt.float32,
                             kind="ExternalInput")
    out_t = nc.dram_tensor("out", [P, 1], mybir.dt.float32,
                           kind="ExternalOutput")

    with TileContext(nc) as tc:
        with (
            tc.tile_pool(name="misc", bufs=1) as misc,
            tc.tile_pool(name="psum", bufs=1, space="PSUM") as psum,
            tc.tile_pool(name="dram", bufs=1, space="DRAM") as dram,
        ):
            u_sb = misc.tile([P, TILES_PER_CORE], mybir.dt.float32)
            nc.sync.dma_start(u_sb[:], u_in[:, :])
            w_sb = misc.tile([P, L * D], mybir.dt.float32)
            nc.sync.dma_start(w_sb[:], w_in[:, :])
            bias_sb = misc.tile([P, 1], mybir.dt.float32)
            nc.sync.dma_start(bias_sb[:], bias_in[:, :])

            # x shard resident in SBUF (50KB/partition) as one tile per DMA
            # chunk, so Tile's per-tensor deps let chunk-k matmuls start
            # while chunk k+1 is still streaming from HBM
            chunk_tiles = 14
            bounds = list(range(0, TILES_PER_CORE, chunk_tiles)) + [TILES_PER_CORE]
            x_chunks = []
            for ci, (lo, hi) in enumerate(zip(bounds[:-1], bounds[1:])):
                xc = misc.tile([P, (hi - lo) * D], mybir.dt.float32, tag=f"xc{ci}")
                nc.sync.dma_start(xc[:], x_in[:, lo * D:hi * D])
                x_chunks.append((lo, hi, xc))

            # z = sum_t x_tile^T @ u0_tile  -> [D, 1] accumulated in PSUM
            z_ps = psum.tile([P, 1], mybir.dt.float32)
            for lo, hi, xc in x_chunks:
                for t in range(lo, hi):
                    nc.tensor.matmul(
                        z_ps[:], lhsT=xc[:, (t - lo) * D:(t - lo + 1) * D],
                        rhs=u_sb[:, t:t + 1],
                        start=(t == 0), stop=(t == TILES_PER_CORE - 1),
                    )

            z_sb = misc.tile([P, 1], mybir.dt.float32)
            nc.vector.tensor_copy(z_sb[:], z_ps[:])

            # AllReduce partial z across the 8 cores
            cc_in = dram.tile([P, 1], mybir.dt.float32)
            cc_out = dram.tile([P, 1], mybir.dt.float32)
            nc.gpsimd.dma_start(cc_in[:], z_sb[:])
            nc.gpsimd.collective_compute(
                "AllReduce", mybir.AluOpType.add,
                replica_groups=[list(range(N_CORES))],
                ins=[cc_in.opt()], outs=[cc_out.opt()],
            )
            v_sb = misc.tile([P, 1], mybir.dt.float32)
            nc.gpsimd.dma_start(v_sb[:], cc_out[:])

            # chain: v <- W_l^T v   (gives (z W0 W1 W2)^T), biases folded on host
            cur = v_sb
            for l in range(L):
                ps = psum.tile([P, 1], mybir.dt.float32, tag=f"c{l}")
                nc.tensor.matmul(ps[:], lhsT=w_sb[:, l * D:(l + 1) * D],
                                 rhs=cur[:], start=True, stop=True)
                nxt = misc.tile([P, 1], mybir.dt.float32, tag=f"v{l}")
                nc.vector.tensor_copy(nxt[:], ps[:])
                cur = nxt

            res = misc.tile([P, 1], mybir.dt.float32)
            nc.vector.tensor_add(res[:], cur[:], bias_sb[:])
            nc.sync.dma_start(out_t[:, :], res[:])
    nc.compile()
    return nc


def kernel(edge_index, node_features, Ws, bs):
    from concourse.bass_utils import run_bass_kernel_spmd

    edge_index = np.asarray(edge_index)
    x = np.asarray(node_features, dtype=np.float32)
    Ws = np.asarray(Ws, dtype=np.float32)
    bs = np.asarray(bs, dtype=np.float32)
    src = edge_index[0].astype(np.int64)
    dst = edge_index[1].astype(np.int64)
    n = x.shape[0]

    # ---- host: index-derived propagation weights (scalar per node) ----
    deg = (np.bincount(dst, minlength=n) + 1.0).astype(np.float32)
    dinv = (1.0 / np.sqrt(deg)).astype(np.float32)
    u = np.ones(n, np.float32)
    sums = []
    for _ in range(L):
        t = dinv * u
        u = dinv * (np.bincount(src, weights=t[dst], minlength=n)
                    .astype(np.float32) + t)
        sums.append(float(u.sum()))
    s2, s1 = sums[0], sums[1]
    u0 = u
    sqrt_n = np.sqrt(np.float32(n))

    # bias terms of the backward-substituted output (zero for zero biases)
    bias_total = ((s1 * bs[0]) @ Ws[1] @ Ws[2] + s2 * bs[1] @ Ws[2]
                  + n * bs[2]) / sqrt_n

    # ---- shard inputs across the 8 cores (one vectorized pass) ----
    u0_pad = np.zeros(N_PAD, np.float32)
    u0_pad[:n] = u0 / sqrt_n
    x_pad = np.empty((N_PAD, D), np.float32)
    x_pad[:n] = x
    x_pad[n:] = 0.0

    u_all = np.ascontiguousarray(
        u0_pad.reshape(N_CORES, TILES_PER_CORE, P).transpose(0, 2, 1))
    x_all = np.ascontiguousarray(
        x_pad.reshape(N_CORES, TILES_PER_CORE, P, D)
        .transpose(0, 2, 1, 3)
        .reshape(N_CORES, P, TILES_PER_CORE * D))

    w_chain = np.ascontiguousarray(
        np.concatenate([Ws[l] for l in range(L)], axis=1))  # [128, 3*128]
    bias_col = np.ascontiguousarray(bias_total.reshape(P, 1))
    in_maps = [{
        "x_sh": x_all[c],
        "u_sh": u_all[c],
        "w_chain": w_chain,
        "bias_tot": bias_col,
    } for c in range(N_CORES)]

    if "nc" not in _CACHE:
        _CACHE["nc"] = _build_nc()
    res = run_bass_kernel_spmd(_CACHE["nc"], in_maps, list(range(N_CORES)))
    return res.results[0]["out"].reshape(D).astype(np.float32)
